# revision 1
# baseline (speedup 1.0000x reference)
"""RNN-T Joiner kernel for 8x TRN2 NeuronCores (Bass/Tile).

out[b,t,u,v] = (enc[b,t]@W_enc.T + b_enc) @ W1.T
            + (pred[b,u]@W_pred.T + b_pred) @ W2.T + b_out
with W1 = W_out[:, :J], W2 = W_out[:, J:].

Strategy: data-parallel over batch (B=8 == n_cores). All biases fold into a
single vector c[v] = W1@b_enc + W2@b_pred + b_out (host-side). Per core:
  S1: E^T[j,t] and P^T[j,u] via PE matmuls (inputs host-transposed).
  S2: Ev[t,v] = E@W1.T  (SBUF, 1MB), Pb[u,v] = P@W2.T + c (K=1 ones-matmul
      folds c into the same PSUM accumulation group).
  S3 (the 64MB output): per (t-block of 128, u): PE identity-matmul copies
      Ev into PSUM (start), K=1 ones-matmul accumulates broadcast Pb[u]
      (stop); ACT/DVE alternately copy PSUM->SBUF out tile; 4MB DMAs with
      32KB-contiguous per-partition chunks stream to HBM.
"""

import numpy as np

ENC_DIM, DEC_DIM, J, V = 512, 640, 512, 1024
B, T, U = 8, 256, 64
N_CORES = 8
UBLK = 8  # u-values per output tile / DMA (tile [128, UBLK*1024] = 4MB DMA)

_CACHE: dict = {}


def _ensure_path():
    try:
        import concourse.bass  # noqa: F401
    except ImportError:
        import sys

        for p in ("/opt/trn_rl_repo", "/root/.axon_site/_ro/trn_rl_repo"):
            if p not in sys.path:
                sys.path.insert(0, p)


def _build_nc():
    import concourse.mybir as mybir
    from concourse import bacc
    from concourse.masks import make_identity
    from concourse.tile import TileContext

    f32 = mybir.dt.float32
    f32r = mybir.dt.float32r  # full-rate fp32 matmul mode (1 cyc/row @ N>=256)
    nc = bacc.Bacc("TRN2", target_bir_lowering=False, debug=False,
                   num_devices=N_CORES)

    encT_d = nc.dram_tensor("encT", [ENC_DIM, T], f32, kind="ExternalInput")
    predT_d = nc.dram_tensor("predT", [DEC_DIM, U], f32, kind="ExternalInput")
    wencT_d = nc.dram_tensor("w_encT", [ENC_DIM, J], f32, kind="ExternalInput")
    wpredT_d = nc.dram_tensor("w_predT", [DEC_DIM, J], f32, kind="ExternalInput")
    w1T_d = nc.dram_tensor("w1T", [J, V], f32, kind="ExternalInput")
    w2T_d = nc.dram_tensor("w2T", [J, V], f32, kind="ExternalInput")
    cvec_d = nc.dram_tensor("cvec", [1, V], f32, kind="ExternalInput")
    out_d = nc.dram_tensor("out", [T, U * V], f32, kind="ExternalOutput")

    NJ = J // 128   # 4 j-chunks
    NE = ENC_DIM // 128  # 4
    ND = DEC_DIM // 128  # 5
    NT = T // 128   # 2 t-blocks
    NV = V // 512   # 2 psum-bank v-chunks

    with TileContext(nc) as tc:
        with (
            tc.tile_pool(name="const", bufs=1) as const,
            tc.tile_pool(name="otile", bufs=2) as opool,
            tc.tile_pool(name="ps_small", bufs=2, space="PSUM") as ps_s,
            tc.tile_pool(name="ps_big", bufs=3, space="PSUM") as ps_b,
        ):
            ident = const.tile([128, 128], f32, tag="ident", name="ident")
            make_identity(nc, ident[:, :])
            ones = const.tile([1, 128], f32, tag="ones", name="ones")
            nc.gpsimd.memset(ones[:, :], 1.0)
            # sel[k, u*128+m] = 1 if k == u else 0: sel[:, u*128:(u+1)*128] is
            # the lhsT that broadcasts Pb row u across all 128 out partitions.
            sel = const.tile([U, U * 128], f32, tag="sel", name="sel")
            nc.gpsimd.memset(sel[:, :], 0.0)
            nc.gpsimd.affine_select(
                out=sel[:, :].rearrange("p (u m) -> p u m", m=128),
                in_=sel[:, :].rearrange("p (u m) -> p u m", m=128),
                compare_op=mybir.AluOpType.not_equal,
                fill=1.0,
                base=0,
                pattern=[[-1, U], [0, 128]],
                channel_multiplier=1,
            )
            cvec = const.tile([1, V], f32, tag="cvec", name="cvec")
            nc.sync.dma_start(cvec[:, :], cvec_d.ap()[:, :])

            def load(tag, dram, rows, cols, nchunks):
                ts = []
                for c in range(nchunks):
                    t = const.tile([128, cols], f32, tag=f"{tag}{c}", name=f"{tag}{c}")
                    nc.sync.dma_start(t[:, :], dram.ap()[c * 128:(c + 1) * 128, :])
                    ts.append(t)
                return ts

            wenc = load("wenc", wencT_d, ENC_DIM, J, NE)
            wpred = load("wpred", wpredT_d, DEC_DIM, J, ND)
            w1 = load("w1_", w1T_d, J, V, NJ)
            w2 = load("w2_", w2T_d, J, V, NJ)
            encs = load("enc", encT_d, ENC_DIM, T, NE)
            preds = load("pred", predT_d, DEC_DIM, U, ND)

            # S1a: E^T[j,t] in 4 chunks of [128, 256]
            ET = []
            for m in range(NJ):
                ps = ps_s.tile([128, T], f32, tag="s1", name="ps1")
                for c in range(NE):
                    nc.tensor.matmul(ps[:, :], lhsT=wenc[c][:, m * 128:(m + 1) * 128],
                                     rhs=encs[c][:, :], start=(c == 0), stop=(c == NE - 1))
                t = const.tile([128, T], f32, tag=f"ET{m}", name=f"ET{m}")
                nc.scalar.copy(t[:, :], ps[:, :])
                ET.append(t)

            # S1b: P^T[j,u] in 4 chunks of [128, 64]
            PT = []
            for m in range(NJ):
                ps = ps_s.tile([128, U], f32, tag="s1", name="ps1")
                for c in range(ND):
                    nc.tensor.matmul(ps[:, :], lhsT=wpred[c][:, m * 128:(m + 1) * 128],
                                     rhs=preds[c][:, :], start=(c == 0), stop=(c == ND - 1))
                t = const.tile([128, U], f32, tag=f"PT{m}", name=f"PT{m}")
                nc.scalar.copy(t[:, :], ps[:, :])
                PT.append(t)

            # S2a: Ev[tb] = E @ W1.T  -> [128, 1024] per t-block
            Ev = [const.tile([128, V], f32, tag=f"Ev{tb}", name=f"Ev{tb}") for tb in range(NT)]
            for tb in range(NT):
                for vb in range(NV):
                    ps = ps_s.tile([128, 512], f32, tag="s1", name="ps1")
                    for m in range(NJ):
                        nc.tensor.matmul(ps[:, :], lhsT=ET[m][:, tb * 128:(tb + 1) * 128],
                                         rhs=w1[m][:, vb * 512:(vb + 1) * 512],
                                         start=(m == 0), stop=(m == NJ - 1))
                    nc.scalar.copy(Ev[tb][:, vb * 512:(vb + 1) * 512], ps[:, :])

            # S2b: Pb = P @ W2.T + c -> [64, 1024]  (c folded via K=1 matmul)
            Pb = const.tile([U, V], f32, tag="Pb", name="Pb")
            for vb in range(NV):
                ps = ps_s.tile([U, 512], f32, tag="s1", name="ps1")
                for m in range(NJ):
                    nc.tensor.matmul(ps[:, :], lhsT=PT[m][:, :],
                                     rhs=w2[m][:, vb * 512:(vb + 1) * 512],
                                     start=(m == 0), stop=False)
                nc.tensor.matmul(ps[:, :], lhsT=ones[:, :U],
                                 rhs=cvec[:, vb * 512:(vb + 1) * 512],
                                 start=False, stop=True)
                nc.scalar.copy(Pb[:, vb * 512:(vb + 1) * 512], ps[:, :])

            # S3: out[t, u, :] = Ev[t, :] + Pb[u, :]
            k = 0
            for tb in range(NT):
                for ug in range(U // UBLK):
                    ot = opool.tile([128, UBLK * V], f32, tag="ot", name="ot")
                    for uu in range(UBLK):
                        u = ug * UBLK + uu
                        ps = ps_b.tile([128, V], f32, tag="s3", name="ps3")
                        for vb in range(NV):
                            sl = slice(vb * 512, (vb + 1) * 512)
                            nc.tensor.matmul(ps[:, sl],
                                             lhsT=ident[:, :],
                                             rhs=Ev[tb][:, sl],
                                             start=True, stop=False)
                            nc.tensor.matmul(ps[:, sl],
                                             lhsT=sel[:, u * 128:(u + 1) * 128],
                                             rhs=Pb[:, sl],
                                             start=False, stop=True)
                        dst = ot[:, uu * V:(uu + 1) * V]
                        if k % 2 == 0:
                            nc.scalar.copy(dst, ps[:, :])
                        else:
                            nc.vector.tensor_copy(dst, ps[:, :])
                        k += 1
                    nc.sync.dma_start(
                        out_d.ap()[tb * 128:(tb + 1) * 128,
                                   ug * UBLK * V:(ug + 1) * UBLK * V],
                        ot[:, :])
    nc.compile()
    return nc


def _get_nc():
    if "nc" not in _CACHE:
        _ensure_path()
        _CACHE["nc"] = _build_nc()
    return _CACHE["nc"]


def _prep_in_maps(enc_out, pred_out, W_enc, b_enc, W_pred, b_pred, W_out, b_out):
    f = np.float32
    enc_out = np.asarray(enc_out, f)
    pred_out = np.asarray(pred_out, f)
    W_enc = np.asarray(W_enc, f)
    W_pred = np.asarray(W_pred, f)
    W_out = np.asarray(W_out, f)
    W1, W2 = W_out[:, :J], W_out[:, J:]
    cvec = (W1 @ np.asarray(b_enc, f) + W2 @ np.asarray(b_pred, f)
            + np.asarray(b_out, f)).astype(f)[None, :]
    shared = {
        "w_encT": np.ascontiguousarray(W_enc.T),
        "w_predT": np.ascontiguousarray(W_pred.T),
        "w1T": np.ascontiguousarray(W1.T),
        "w2T": np.ascontiguousarray(W2.T),
        "cvec": cvec,
    }
    return [
        {"encT": np.ascontiguousarray(enc_out[b].T),
         "predT": np.ascontiguousarray(pred_out[b].T), **shared}
        for b in range(B)
    ]


def run(in_maps, trace=False, **kw):
    _ensure_path()
    from concourse.bass_utils import run_bass_kernel_spmd

    return run_bass_kernel_spmd(_get_nc(), in_maps, list(range(N_CORES)),
                                trace=trace, **kw)


def kernel(enc_out, pred_out, W_enc, b_enc, W_pred, b_pred, W_out, b_out):
    in_maps = _prep_in_maps(enc_out, pred_out, W_enc, b_enc, W_pred, b_pred,
                            W_out, b_out)
    res = run(in_maps, trace=False)
    return np.stack([r["out"].reshape(T, U, V) for r in res.results], axis=0)



# revision 5
# speedup vs baseline: 3.9012x; 3.9012x over previous
"""RNN-T Joiner kernel for 8x TRN2 NeuronCores (Bass/Tile).

out[b,t,u,v] = (enc[b,t]@W_enc.T + b_enc) @ W1.T
            + (pred[b,u]@W_pred.T + b_pred) @ W2.T + b_out
with W1 = W_out[:, :J], W2 = W_out[:, J:].

Strategy: data-parallel over batch (B=8 == n_cores). All biases fold into a
single vector c[v] = W1@b_enc + W2@b_pred + b_out (host-side). Per core:
  S1: E^T[j,t] and P^T[j,u] via PE matmuls (inputs host-transposed).
  S2: Ev[t,v] = E@W1.T  (SBUF), Pb[u,v] = P@W2.T + c (K=1 ones-matmul
      folds c into the same PSUM accumulation group).
  S3 (the output): per (t-block of 128, u): a selector matmul broadcasts
      Pb[u] across all 128 partitions into PSUM. Two alternating paths:
        A: PE identity-matmul accumulates Ev into the same PSUM group,
           ACT copies PSUM -> bf16 out tile.
        D: DVE tensor_tensor adds Ev (SBUF) to PSUM, writing bf16 out tile.
All matmul operands are bf16 (1 cyc/col on the PE, FWL weight loads);
accumulation stays fp32 in PSUM. Output is written to HBM as bf16 (halves
write traffic; tolerance is 2e-2) and upcast to f32 on host.
"""

import numpy as np

ENC_DIM, DEC_DIM, J, V = 512, 640, 512, 1024
B, T, U = 8, 256, 64
N_CORES = 8
UBLK = 8  # u-values per output tile / DMA ([128, UBLK*1024] bf16 = 2MB DMA)

_CACHE: dict = {}


def _ensure_path():
    try:
        import concourse.bass  # noqa: F401
    except ImportError:
        import sys

        for p in ("/opt/trn_rl_repo", "/root/.axon_site/_ro/trn_rl_repo"):
            if p not in sys.path:
                sys.path.insert(0, p)


def _build_nc():
    import concourse.mybir as mybir
    from concourse import bacc
    from concourse.masks import make_identity
    from concourse.tile import TileContext

    f32 = mybir.dt.float32
    bf16 = mybir.dt.bfloat16
    nc = bacc.Bacc("TRN2", target_bir_lowering=False, debug=False,
                   num_devices=N_CORES)

    encT_d = nc.dram_tensor("encT", [ENC_DIM, T], bf16, kind="ExternalInput")
    predT_d = nc.dram_tensor("predT", [DEC_DIM, U], bf16, kind="ExternalInput")
    wencT_d = nc.dram_tensor("w_encT", [ENC_DIM, J], bf16, kind="ExternalInput")
    wpredT_d = nc.dram_tensor("w_predT", [DEC_DIM, J], bf16, kind="ExternalInput")
    w1T_d = nc.dram_tensor("w1T", [J, V], bf16, kind="ExternalInput")
    w2T_d = nc.dram_tensor("w2T", [J, V], bf16, kind="ExternalInput")
    cvec_d = nc.dram_tensor("cvec", [1, V], bf16, kind="ExternalInput")
    out_d = nc.dram_tensor("out", [T, U * V], bf16, kind="ExternalOutput")

    NJ = J // 128   # 4 j-chunks
    NE = ENC_DIM // 128  # 4
    ND = DEC_DIM // 128  # 5
    NT = T // 128   # 2 t-blocks
    NV = V // 512   # 2 psum-bank v-chunks

    with TileContext(nc) as tc:
        with (
            tc.tile_pool(name="const", bufs=1) as const,
            tc.tile_pool(name="otile", bufs=3) as opool,
            tc.tile_pool(name="ps_small", bufs=2, space="PSUM") as ps_s,
            tc.tile_pool(name="ps_big", bufs=3, space="PSUM") as ps_b,
        ):
            ident = const.tile([128, 128], bf16, tag="ident", name="ident")
            make_identity(nc, ident[:, :])
            ones = const.tile([1, 128], bf16, tag="ones", name="ones")
            nc.gpsimd.memset(ones[:, :], 1.0)
            # sel[k, u*128+m] = 1 if k == u else 0: sel[:, u*128:(u+1)*128] is
            # the lhsT that broadcasts Pb row u across all 128 out partitions.
            sel = const.tile([U, U * 128], bf16, tag="sel", name="sel")
            nc.gpsimd.memset(sel[:, :], 0.0)
            nc.gpsimd.affine_select(
                out=sel[:, :].rearrange("p (u m) -> p u m", m=128),
                in_=sel[:, :].rearrange("p (u m) -> p u m", m=128),
                compare_op=mybir.AluOpType.not_equal,
                fill=1.0,
                base=0,
                pattern=[[-1, U], [0, 128]],
                channel_multiplier=1,
            )
            cvec = const.tile([1, V], bf16, tag="cvec", name="cvec")
            nc.sync.dma_start(cvec[:, :], cvec_d.ap()[:, :])

            def load(tag, dram, rows, cols, nchunks):
                ts = []
                for c in range(nchunks):
                    t = const.tile([128, cols], bf16, tag=f"{tag}{c}", name=f"{tag}{c}")
                    nc.sync.dma_start(t[:, :], dram.ap()[c * 128:(c + 1) * 128, :])
                    ts.append(t)
                return ts

            wenc = load("wenc", wencT_d, ENC_DIM, J, NE)
            wpred = load("wpred", wpredT_d, DEC_DIM, J, ND)
            w1 = load("w1_", w1T_d, J, V, NJ)
            w2 = load("w2_", w2T_d, J, V, NJ)
            encs = load("enc", encT_d, ENC_DIM, T, NE)
            preds = load("pred", predT_d, DEC_DIM, U, ND)

            # S1a: E^T[j,t] in 4 chunks of [128, 256]
            ET = []
            for m in range(NJ):
                ps = ps_s.tile([128, T], f32, tag="s1", name="ps1")
                for c in range(NE):
                    nc.tensor.matmul(ps[:, :], lhsT=wenc[c][:, m * 128:(m + 1) * 128],
                                     rhs=encs[c][:, :], start=(c == 0), stop=(c == NE - 1))
                t = const.tile([128, T], bf16, tag=f"ET{m}", name=f"ET{m}")
                nc.scalar.copy(t[:, :], ps[:, :])
                ET.append(t)

            # S1b: P^T[j,u] in 4 chunks of [128, 64]
            PT = []
            for m in range(NJ):
                ps = ps_s.tile([128, U], f32, tag="s1", name="ps1")
                for c in range(ND):
                    nc.tensor.matmul(ps[:, :], lhsT=wpred[c][:, m * 128:(m + 1) * 128],
                                     rhs=preds[c][:, :], start=(c == 0), stop=(c == ND - 1))
                t = const.tile([128, U], bf16, tag=f"PT{m}", name=f"PT{m}")
                nc.scalar.copy(t[:, :], ps[:, :])
                PT.append(t)

            # S2a: Ev[tb] = E @ W1.T  -> [128, 1024] per t-block
            Ev = [const.tile([128, V], bf16, tag=f"Ev{tb}", name=f"Ev{tb}") for tb in range(NT)]
            for tb in range(NT):
                for vb in range(NV):
                    ps = ps_s.tile([128, 512], f32, tag="s1", name="ps1")
                    for m in range(NJ):
                        nc.tensor.matmul(ps[:, :], lhsT=ET[m][:, tb * 128:(tb + 1) * 128],
                                         rhs=w1[m][:, vb * 512:(vb + 1) * 512],
                                         start=(m == 0), stop=(m == NJ - 1))
                    nc.scalar.copy(Ev[tb][:, vb * 512:(vb + 1) * 512], ps[:, :])

            # S2b: Pb = P @ W2.T + c -> [64, 1024]  (c folded via K=1 matmul)
            Pb = const.tile([U, V], bf16, tag="Pb", name="Pb")
            for vb in range(NV):
                ps = ps_s.tile([U, 512], f32, tag="s1", name="ps1")
                for m in range(NJ):
                    nc.tensor.matmul(ps[:, :], lhsT=PT[m][:, :],
                                     rhs=w2[m][:, vb * 512:(vb + 1) * 512],
                                     start=(m == 0), stop=False)
                nc.tensor.matmul(ps[:, :], lhsT=ones[:, :U],
                                 rhs=cvec[:, vb * 512:(vb + 1) * 512],
                                 start=False, stop=True)
                nc.scalar.copy(Pb[:, vb * 512:(vb + 1) * 512], ps[:, :])

            # S3: out[t, u, :] = Ev[t, :] + Pb[u, :]
            for tb in range(NT):
                for ug in range(U // UBLK):
                    ot = opool.tile([128, UBLK * V], bf16, tag="ot", name="ot")
                    for uu in range(UBLK):
                        u = ug * UBLK + uu
                        path_a = (uu % 2 == 0)
                        ps = ps_b.tile([128, V], f32, tag="s3", name="ps3")
                        for vb in range(NV):
                            sl = slice(vb * 512, (vb + 1) * 512)
                            # broadcast Pb[u] across all 128 partitions (K=64)
                            nc.tensor.matmul(ps[:, sl],
                                             lhsT=sel[:, u * 128:(u + 1) * 128],
                                             rhs=Pb[:, sl],
                                             start=True, stop=not path_a)
                            if path_a:
                                nc.tensor.matmul(ps[:, sl],
                                                 lhsT=ident[:, :],
                                                 rhs=Ev[tb][:, sl],
                                                 start=False, stop=True)
                        dst = ot[:, uu * V:(uu + 1) * V]
                        if path_a:
                            nc.scalar.copy(dst, ps[:, :])
                        else:
                            nc.vector.tensor_tensor(dst, ps[:, :], Ev[tb][:, :],
                                                    op=mybir.AluOpType.add)
                    nc.sync.dma_start(
                        out_d.ap()[tb * 128:(tb + 1) * 128,
                                   ug * UBLK * V:(ug + 1) * UBLK * V],
                        ot[:, :])
    nc.compile()
    return nc


def _get_nc():
    if "nc" not in _CACHE:
        _ensure_path()
        _CACHE["nc"] = _build_nc()
    return _CACHE["nc"]


def _prep_in_maps(enc_out, pred_out, W_enc, b_enc, W_pred, b_pred, W_out, b_out):
    import ml_dtypes

    f = np.float32
    bf = ml_dtypes.bfloat16
    enc_out = np.asarray(enc_out, f)
    pred_out = np.asarray(pred_out, f)
    W_enc = np.asarray(W_enc, f)
    W_pred = np.asarray(W_pred, f)
    W_out = np.asarray(W_out, f)
    W1, W2 = W_out[:, :J], W_out[:, J:]
    cvec = (W1 @ np.asarray(b_enc, f) + W2 @ np.asarray(b_pred, f)
            + np.asarray(b_out, f)).astype(f)[None, :]
    shared = {
        "w_encT": np.ascontiguousarray(W_enc.T).astype(bf),
        "w_predT": np.ascontiguousarray(W_pred.T).astype(bf),
        "w1T": np.ascontiguousarray(W1.T).astype(bf),
        "w2T": np.ascontiguousarray(W2.T).astype(bf),
        "cvec": cvec.astype(bf),
    }
    return [
        {"encT": np.ascontiguousarray(enc_out[b].T).astype(bf),
         "predT": np.ascontiguousarray(pred_out[b].T).astype(bf), **shared}
        for b in range(B)
    ]


def run(in_maps, trace=False, **kw):
    _ensure_path()
    from concourse.bass_utils import run_bass_kernel_spmd

    return run_bass_kernel_spmd(_get_nc(), in_maps, list(range(N_CORES)),
                                trace=trace, **kw)


def kernel(enc_out, pred_out, W_enc, b_enc, W_pred, b_pred, W_out, b_out):
    in_maps = _prep_in_maps(enc_out, pred_out, W_enc, b_enc, W_pred, b_pred,
                            W_out, b_out)
    res = run(in_maps, trace=False)
    return np.stack([np.asarray(r["out"]).astype(np.float32).reshape(T, U, V)
                     for r in res.results], axis=0)


# revision 6
# speedup vs baseline: 4.8515x; 1.2436x over previous
"""RNN-T Joiner kernel for 8x TRN2 NeuronCores (Bass/Tile).

out[b,t,u,v] = (enc[b,t]@W_enc.T + b_enc) @ W1.T
            + (pred[b,u]@W_pred.T + b_pred) @ W2.T + b_out
with W1 = W_out[:, :J], W2 = W_out[:, J:].

Strategy: data-parallel over batch (B=8 == n_cores). All biases fold into a
single vector c[v] = W1@b_enc + W2@b_pred + b_out (host-side). Per core:
  S1: E^T[j,t] and P^T[j,u] via PE matmuls (inputs host-transposed).
  S2: Ev[t,v] = E@W1.T  (SBUF), Pb[u,v] = P@W2.T + c (K=1 ones-matmul
      folds c into the same PSUM accumulation group).
  S3 (the output): out[t, u, :] = Ev[t, :] + Pb[u, :].
      First UBLK u-values (per t-block) go through the PE: selector-matmul
      broadcasts Pb[u] into PSUM, identity-matmul accumulates Ev, ACT
      copies PSUM -> bf16 out tile. This fills the pipe while Pbrep builds.
      Remaining u: Pb rows are pre-broadcast ("Pbrep" groups of UBLK u,
      double-buffered: sel-matmul -> PSUM -> ACT copy -> bf16 SBUF); the
      add is then a pure-SBUF bf16 DVE tensor_tensor (2x mode, ~594ns per
      [128,1024] chunk vs ~1224ns with a PSUM operand).
All matmul operands are bf16 (1 cyc/col on the PE, FWL weight loads);
accumulation stays fp32 in PSUM. Output is written to HBM as bf16 (halves
write traffic; tolerance is 2e-2) and upcast to f32 on host.
"""

import numpy as np

ENC_DIM, DEC_DIM, J, V = 512, 640, 512, 1024
B, T, U = 8, 256, 64
N_CORES = 8
UBLK = 4  # u-values per output tile / DMA ([128, UBLK*1024] bf16 = 1MB DMA)
NG = U // UBLK  # 16 u-groups

_CACHE: dict = {}


def _ensure_path():
    try:
        import concourse.bass  # noqa: F401
    except ImportError:
        import sys

        for p in ("/opt/trn_rl_repo", "/root/.axon_site/_ro/trn_rl_repo"):
            if p not in sys.path:
                sys.path.insert(0, p)


def _build_nc():
    import concourse.mybir as mybir
    from concourse import bacc
    from concourse.masks import make_identity
    from concourse.tile import TileContext

    f32 = mybir.dt.float32
    bf16 = mybir.dt.bfloat16
    nc = bacc.Bacc("TRN2", target_bir_lowering=False, debug=False,
                   num_devices=N_CORES)

    encT_d = nc.dram_tensor("encT", [ENC_DIM, T], bf16, kind="ExternalInput")
    predT_d = nc.dram_tensor("predT", [DEC_DIM, U], bf16, kind="ExternalInput")
    wencT_d = nc.dram_tensor("w_encT", [ENC_DIM, J], bf16, kind="ExternalInput")
    wpredT_d = nc.dram_tensor("w_predT", [DEC_DIM, J], bf16, kind="ExternalInput")
    w1T_d = nc.dram_tensor("w1T", [J, V], bf16, kind="ExternalInput")
    w2T_d = nc.dram_tensor("w2T", [J, V], bf16, kind="ExternalInput")
    cvec_d = nc.dram_tensor("cvec", [1, V], bf16, kind="ExternalInput")
    out_d = nc.dram_tensor("out", [T, U * V], bf16, kind="ExternalOutput")

    NJ = J // 128   # 4 j-chunks
    NE = ENC_DIM // 128  # 4
    ND = DEC_DIM // 128  # 5
    NT = T // 128   # 2 t-blocks
    NV = V // 512   # 2 psum-bank v-chunks

    with TileContext(nc) as tc:
        with (
            tc.tile_pool(name="const", bufs=1) as const,
            tc.tile_pool(name="pbrep", bufs=2) as pbpool,
            tc.tile_pool(name="otile", bufs=4) as opool,
            tc.tile_pool(name="ps", bufs=4, space="PSUM") as psp,
        ):
            ident = const.tile([128, 128], bf16, tag="ident", name="ident")
            make_identity(nc, ident[:, :])
            ones = const.tile([1, 128], bf16, tag="ones", name="ones")
            nc.gpsimd.memset(ones[:, :], 1.0)
            # sel[k, u*128+m] = 1 if k == u else 0: sel[:, u*128:(u+1)*128] is
            # the lhsT that broadcasts Pb row u across all 128 out partitions.
            sel = const.tile([U, U * 128], bf16, tag="sel", name="sel")
            nc.gpsimd.memset(sel[:, :], 0.0)
            nc.gpsimd.affine_select(
                out=sel[:, :].rearrange("p (u m) -> p u m", m=128),
                in_=sel[:, :].rearrange("p (u m) -> p u m", m=128),
                compare_op=mybir.AluOpType.not_equal,
                fill=1.0,
                base=0,
                pattern=[[-1, U], [0, 128]],
                channel_multiplier=1,
            )
            cvec = const.tile([1, V], bf16, tag="cvec", name="cvec")
            nc.sync.dma_start(cvec[:, :], cvec_d.ap()[:, :])

            def load(tag, dram, rows, cols, nchunks):
                ts = []
                for c in range(nchunks):
                    t = const.tile([128, cols], bf16, tag=f"{tag}{c}", name=f"{tag}{c}")
                    nc.sync.dma_start(t[:, :], dram.ap()[c * 128:(c + 1) * 128, :])
                    ts.append(t)
                return ts

            wpred = load("wpred", wpredT_d, DEC_DIM, J, ND)
            preds = load("pred", predT_d, DEC_DIM, U, ND)
            wenc = load("wenc", wencT_d, ENC_DIM, J, NE)
            encs = load("enc", encT_d, ENC_DIM, T, NE)
            w2 = load("w2_", w2T_d, J, V, NJ)
            w1 = load("w1_", w1T_d, J, V, NJ)

            # S1b: P^T[j,u] in 4 chunks of [128, 64]
            PT = []
            for m in range(NJ):
                ps = psp.tile([128, V], f32, tag="ps", name="ps")
                for c in range(ND):
                    nc.tensor.matmul(ps[:, :U], lhsT=wpred[c][:, m * 128:(m + 1) * 128],
                                     rhs=preds[c][:, :], start=(c == 0), stop=(c == ND - 1))
                t = const.tile([128, U], bf16, tag=f"PT{m}", name=f"PT{m}")
                nc.scalar.copy(t[:, :], ps[:, :U])
                PT.append(t)

            # S2b: Pb = P @ W2.T + c -> [64, 1024]  (c folded via K=1 matmul)
            Pb = const.tile([U, V], bf16, tag="Pb", name="Pb")
            for vb in range(NV):
                ps = psp.tile([128, V], f32, tag="ps", name="ps")
                for m in range(NJ):
                    nc.tensor.matmul(ps[:U, :512], lhsT=PT[m][:, :],
                                     rhs=w2[m][:, vb * 512:(vb + 1) * 512],
                                     start=(m == 0), stop=False)
                nc.tensor.matmul(ps[:U, :512], lhsT=ones[:, :U],
                                 rhs=cvec[:, vb * 512:(vb + 1) * 512],
                                 start=False, stop=True)
                nc.scalar.copy(Pb[:, vb * 512:(vb + 1) * 512], ps[:U, :512])

            # S1a: E^T[j,t] in 4 chunks of [128, 256]
            ET = []
            for m in range(NJ):
                ps = psp.tile([128, V], f32, tag="ps", name="ps")
                for c in range(NE):
                    nc.tensor.matmul(ps[:, :T], lhsT=wenc[c][:, m * 128:(m + 1) * 128],
                                     rhs=encs[c][:, :], start=(c == 0), stop=(c == NE - 1))
                t = const.tile([128, T], bf16, tag=f"ET{m}", name=f"ET{m}")
                nc.scalar.copy(t[:, :], ps[:, :T])
                ET.append(t)

            # S2a: Ev[tb] = E @ W1.T  -> [128, 1024] per t-block
            Ev = [const.tile([128, V], bf16, tag=f"Ev{tb}", name=f"Ev{tb}") for tb in range(NT)]
            for tb in range(NT):
                for vb in range(NV):
                    ps = psp.tile([128, V], f32, tag="ps", name="ps")
                    for m in range(NJ):
                        nc.tensor.matmul(ps[:, :512], lhsT=ET[m][:, tb * 128:(tb + 1) * 128],
                                         rhs=w1[m][:, vb * 512:(vb + 1) * 512],
                                         start=(m == 0), stop=(m == NJ - 1))
                    nc.scalar.copy(Ev[tb][:, vb * 512:(vb + 1) * 512], ps[:, :512])

            def build_pbrep(g):
                """Pre-broadcast Pb rows u=g*UBLK..g*UBLK+UBLK-1 across all
                128 partitions: [128, UBLK*1024] bf16."""
                rep = pbpool.tile([128, UBLK * V], bf16, tag="rep", name="rep")
                for uu in range(UBLK):
                    u = g * UBLK + uu
                    ps = psp.tile([128, V], f32, tag="ps", name="ps")
                    for vb in range(NV):
                        sl = slice(vb * 512, (vb + 1) * 512)
                        nc.tensor.matmul(ps[:, sl],
                                         lhsT=sel[:, u * 128:(u + 1) * 128],
                                         rhs=Pb[:, sl],
                                         start=True, stop=True)
                    nc.scalar.copy(rep[:, uu * V:(uu + 1) * V], ps[:, :])
                return rep

            # S3 group 0 via the PE/ACT path (while Pbrep for group 1 builds)
            rep_next = build_pbrep(1)
            for tb in range(NT):
                ot = opool.tile([128, UBLK * V], bf16, tag="ot", name="ot")
                for uu in range(UBLK):
                    u = uu
                    ps = psp.tile([128, V], f32, tag="ps", name="ps")
                    for vb in range(NV):
                        sl = slice(vb * 512, (vb + 1) * 512)
                        nc.tensor.matmul(ps[:, sl],
                                         lhsT=sel[:, u * 128:(u + 1) * 128],
                                         rhs=Pb[:, sl],
                                         start=True, stop=False)
                        nc.tensor.matmul(ps[:, sl],
                                         lhsT=ident[:, :],
                                         rhs=Ev[tb][:, sl],
                                         start=False, stop=True)
                    nc.scalar.copy(ot[:, uu * V:(uu + 1) * V], ps[:, :])
                nc.sync.dma_start(
                    out_d.ap()[tb * 128:(tb + 1) * 128, :UBLK * V], ot[:, :])

            # S3 groups 1..NG-1 via the DVE SBUF-SBUF path
            for g in range(1, NG):
                rep = rep_next
                if g + 1 < NG:
                    rep_next = build_pbrep(g + 1)
                for tb in range(NT):
                    ot = opool.tile([128, UBLK * V], bf16, tag="ot", name="ot")
                    for uu in range(UBLK):
                        nc.vector.tensor_tensor(
                            ot[:, uu * V:(uu + 1) * V],
                            Ev[tb][:, :],
                            rep[:, uu * V:(uu + 1) * V],
                            op=mybir.AluOpType.add)
                    nc.sync.dma_start(
                        out_d.ap()[tb * 128:(tb + 1) * 128,
                                   g * UBLK * V:(g + 1) * UBLK * V],
                        ot[:, :])
    nc.compile()
    return nc


def _get_nc():
    if "nc" not in _CACHE:
        _ensure_path()
        _CACHE["nc"] = _build_nc()
    return _CACHE["nc"]


def _prep_in_maps(enc_out, pred_out, W_enc, b_enc, W_pred, b_pred, W_out, b_out):
    import ml_dtypes

    f = np.float32
    bf = ml_dtypes.bfloat16
    enc_out = np.asarray(enc_out, f)
    pred_out = np.asarray(pred_out, f)
    W_enc = np.asarray(W_enc, f)
    W_pred = np.asarray(W_pred, f)
    W_out = np.asarray(W_out, f)
    W1, W2 = W_out[:, :J], W_out[:, J:]
    cvec = (W1 @ np.asarray(b_enc, f) + W2 @ np.asarray(b_pred, f)
            + np.asarray(b_out, f)).astype(f)[None, :]
    shared = {
        "w_encT": np.ascontiguousarray(W_enc.T).astype(bf),
        "w_predT": np.ascontiguousarray(W_pred.T).astype(bf),
        "w1T": np.ascontiguousarray(W1.T).astype(bf),
        "w2T": np.ascontiguousarray(W2.T).astype(bf),
        "cvec": cvec.astype(bf),
    }
    return [
        {"encT": np.ascontiguousarray(enc_out[b].T).astype(bf),
         "predT": np.ascontiguousarray(pred_out[b].T).astype(bf), **shared}
        for b in range(B)
    ]


def run(in_maps, trace=False, **kw):
    _ensure_path()
    from concourse.bass_utils import run_bass_kernel_spmd

    return run_bass_kernel_spmd(_get_nc(), in_maps, list(range(N_CORES)),
                                trace=trace, **kw)


def kernel(enc_out, pred_out, W_enc, b_enc, W_pred, b_pred, W_out, b_out):
    in_maps = _prep_in_maps(enc_out, pred_out, W_enc, b_enc, W_pred, b_pred,
                            W_out, b_out)
    res = run(in_maps, trace=False)
    return np.stack([np.asarray(r["out"]).astype(np.float32).reshape(T, U, V)
                     for r in res.results], axis=0)
